# revision 25
# baseline (speedup 1.0000x reference)
"""Trainium2 Bass kernel for nn_Encoder (dense MLP 6->8->4->2->1 + softplus).

Pure data parallel over 8 NeuronCores. v2: every layer runs as an fp8e4m3
DoubleRow matmul (2 output cols/cycle, K-pairs contracted together), which
cuts PE matmul work from ~92k to ~31k cycles/core. All inter-layer
activations are fp8e4m3 (numpy-modeled rel_absmax 1.41e-2 vs 2e-2 budget;
HW fp8 rounding verified bit-identical to ml_dtypes in probe1.py).

The critical resource is PSUM->SBUF evacuation: only ACT (1.2 GHz) + DVE
(0.96 GHz) can read PSUM, 1 elem/lane/cycle (GPSIMD cannot access PSUM --
walrus birverifier; bf16 PSUM + DVE 2x is TRN3-only). 61440 evac columns
=> ~32us floor. Ops are sized to amortize per-op overhead (z1+z2 as
[128,1024]) and assigned to ACT/DVE by a build-time balancer.

Schedule (wave w, one 1024-group-col chunk per wave): L1(w) -> E1(w-1) ->
L2 pair k at 2k+4 -> E2 at 2k+5 -> L3 at 2k+6 -> E3 at 2k+7 -> L4 quad q
at 4q+10 -> E4(exp) at 4q+11; every cross-engine edge has >=1 wave slack.
PSUM: z1 [128,1024] double-buffered (4 banks) + z2 [128,1024] (2) +
z3 [128,512] + z4 [128,512] = 8 banks.

The PE HAM clock-gate (1.2 -> 2.4 GHz after ~3.4us of ~95% PE-busy) is
warmed with head matmuls + LDWEIGHTS fillers; with DoubleRow the PE has
so much slack that even a never-warm run stays near the evac floor.

Device computes exp(z4+b9) (no softplus ACT table in this toolchain),
ships bf16; host finishes softplus = log1p(exp) while unscrambling.
"""

import os
import sys

sys.path.insert(0, "/opt/trn_rl_repo")

import numpy as np

import concourse.bass as bass
import concourse.mybir as mybir
import concourse.tile as tile
from concourse.bass_utils import run_bass_kernel_spmd

# ---------------------------------------------------------------- geometry
N_CORES = 8
N_ROWS = 4194304
ROWS_PER_CORE = N_ROWS // N_CORES          # 524288
G = 16                                      # rows per group-column
COLS = ROWS_PER_CORE // G                   # 32768 group-cols per core
BLK = 1024                                  # group-cols per pipeline wave
N_BLK = COLS // BLK                         # 32
FD = 512                                    # matmul free dim / PSUM bank
OUT_COLS = COLS // 8                        # 4096 output columns [128p]
BF16 = mybir.dt.bfloat16
F8 = mybir.dt.float8e4
F32 = mybir.dt.float32

# walrus in this container rejects instructions carrying more than
# _MAX_WAITS sync waits; split the surplus onto same-engine NoOps placed
# immediately before the instruction.
_MAX_WAITS = int(os.environ.get("KMAXW", "1"))

# HAM warmup: the PE clock-gate opens after one 4096-cycle (3.4us @1.2GHz)
# window of ~fully-busy PE. v3 showed a chain with 128-col LDWEIGHTS between
# matmuls (80% duty) does NOT open it; use an 8-col stationary so the chain
# is ~96% matmul (8 b2b 512-col matmuls = 3.4us cold).
# 6 pure warm matmuls (2.6us) + the first real L1's matmuls complete the
# 3.4us window at ~97% duty; the gate opens just as the pipeline starts.
WARMUP = int(os.environ.get("KWARMUP", "6"))
WARMCOLS = int(os.environ.get("KWARMCOLS", "8"))
# PE filler mode: dependency-free LDWEIGHTS keep the PE busy so the HAM
# clock-gate opens. "ldw" / "off". With a >=3.4us back-to-back warmup the
# gate opens at the head and only re-throttles after >=3.4us of contiguous
# PE idle, which the steady-state pipeline never has -- fillers off.
FILL_MODE = os.environ.get("KFILLMODE", "off")
FILL_PER_CHUNK = int(os.environ.get("KFILLN", "2"))


def _split_multi_waits(nc, max_waits=_MAX_WAITS):
    ctr = 0
    for f in nc.m.functions:
        for bb in f.blocks:
            out = []
            for inst in bb.instructions:
                mw = 1 if ("Dma" in inst.opcode or "DMA" in inst.opcode
                           or "Trigger" in inst.opcode) else max_waits
                si = getattr(inst, "sync_info", None)
                if si is not None and si.on_wait and len(si.on_wait) > mw:
                    waits = list(si.on_wait)
                    split = len(waits) - mw
                    for i in range(0, split, max_waits):
                        nop = mybir.InstNoOp(
                            name=f"waitsplit-{ctr}", ins=[], outs=[]
                        )
                        ctr += 1
                        nop.engine = inst.engine
                        nop.sync_info = mybir.SyncInfo(
                            on_wait=waits[i : min(i + max_waits, split)],
                            on_update=[],
                        )
                        out.append(nop)
                    inst.sync_info = mybir.SyncInfo(
                        on_wait=waits[split:], on_update=list(si.on_update)
                    )
                out.append(inst)
            bb.instructions[:] = out


# Set KERNEL_TRACE=1 to neuron-profile the run; kernel() then stashes the
# BassKernelResults (exec_time_ns, trace paths) in LAST_RESULTS.
TRACE = os.environ.get("KERNEL_TRACE", "0") == "1"
LAST_RESULTS = None


def _register_ntff_hook():
    """The image's antenv lacks axon_hooks; inject it and register the ctypes
    NTFF profile hook so run_bass_kernel_spmd(trace=True) works under axon."""
    import types

    if "antenv.axon_hooks" not in sys.modules:
        mod = types.ModuleType("antenv.axon_hooks")
        mod._hook = None

        def set_axon_ntff_profile_hook(h, _mod=mod):
            _mod._hook = h

        def get_axon_ntff_profile_hook(_mod=mod):
            return _mod._hook

        mod.set_axon_ntff_profile_hook = set_axon_ntff_profile_hook
        mod.get_axon_ntff_profile_hook = get_axon_ntff_profile_hook
        sys.modules["antenv.axon_hooks"] = mod
        import antenv

        antenv.axon_hooks = mod
    mod = sys.modules["antenv.axon_hooks"]
    if mod.get_axon_ntff_profile_hook() is None:
        try:
            from trn_agent_boot.trn_boot import _ntff_profile_via_ctypes

            mod.set_axon_ntff_profile_hook(
                _ntff_profile_via_ctypes("/opt/axon/libaxon_pjrt.so")
            )
        except Exception:
            pass


# ---------------------------------------------------------------- program
def build_program(n_blk=N_BLK, split_waits=True, warmup=WARMUP):
    """One SPMD NeuronCore program; all 8 cores run it on their own shard."""
    nc = bass.Bass("TRN2", target_bir_lowering=False, debug=False,
                   num_devices=N_CORES)

    cols = n_blk * BLK
    out_cols = cols // 8
    n_ch = n_blk
    n_pair = n_ch // 2
    n_quad = n_ch // 4

    xb = nc.dram_tensor("xb", [96, cols], F8, kind="ExternalInput").ap()
    # L1 stationary stays bf16: fp8 gains nothing without DoubleRow (the
    # array runs 1 elem/cell/cycle regardless of width) and W1-fp8 is the
    # single biggest error term. Mixed bf16-stationary x fp8-moving keeps
    # FWL on.
    w1b = nc.dram_tensor("w1b", [96, 128], BF16, kind="ExternalInput").ap()
    # weight pack [128, 768] fp8: [:,0:256]=w2d, [:,256:512]=w3d,
    # [:,512:768]=w4d (each [128, 2, 128] DoubleRow pair)
    wp = nc.dram_tensor("wpack", [128, 768], F8, kind="ExternalInput").ap()
    bv = nc.dram_tensor("bvecs", [128, 4], F32, kind="ExternalInput").ap()
    # raw z4 ships as fp16 (rel err 5e-4); host computes the whole
    # softplus(z4+b9). Keeping exp off the device frees ~5us of
    # forced-ACT work into the ACT/DVE balance pool.
    outb = nc.dram_tensor("outb", [128, out_cols], mybir.dt.float16,
                          kind="ExternalOutput").ap()

    Relu = mybir.ActivationFunctionType.Relu
    Copy = mybir.ActivationFunctionType.Copy
    ADD = mybir.AluOpType.add
    MAX = mybir.AluOpType.max
    DR = mybir.MatmulPerfMode.DoubleRow
    F16 = mybir.dt.float16

    # x DMA chunks (in group-cols); first small so compute starts early
    if n_blk >= 16:
        chunks = [(0, 2048), (2048, 8192), (8192, 20480), (20480, cols)]
    else:
        chunks = [(i * BLK, (i + 1) * BLK) for i in range(n_blk)]
    chunk_of_block = []
    for i in range(n_ch):
        for ci, (c0, c1) in enumerate(chunks):
            if c0 <= i * BLK < c1:
                chunk_of_block.append((ci, i * BLK - c0))
                break

    # greedy ACT/DVE load balancer for the PSUM->SBUF evacuation ops.
    # per-op overheads measured from neuron-profile (v1): ACTIVATE +352cyc,
    # TENSOR_SCALAR +209cyc.
    load = {"act": 0.0, "dve": 0.0}
    act_oh = int(os.environ.get("KACTOH", "352"))
    dve_oh = int(os.environ.get("KDVEOH", "209"))

    def evac_cost(n, eng):
        return (n + act_oh) / 1.2 if eng == "act" else (n + dve_oh) / 0.96

    def pick_engine(n):
        a = load["act"] + evac_cost(n, "act")
        d = load["dve"] + evac_cost(n, "dve")
        eng = "act" if a <= d else "dve"
        load[eng] += evac_cost(n, eng)
        return eng

    with tile.TileContext(nc) as tc:
        with (
            tc.tile_pool(name="consts", bufs=1) as cpool,
            tc.tile_pool(name="xin", bufs=len(chunks)) as xpool,
            tc.tile_pool(name="z1r", bufs=4) as z1pool,
            tc.tile_pool(name="z2r", bufs=2) as z2pool,
            tc.tile_pool(name="z3r", bufs=2) as z3pool,
            tc.tile_pool(name="osb", bufs=2) as opool,
            tc.tile_pool(name="ps1", bufs=2, space="PSUM") as ps1,
            tc.tile_pool(name="ps2", bufs=1, space="PSUM") as ps2,
            tc.tile_pool(name="ps3", bufs=1, space="PSUM") as ps3,
            tc.tile_pool(name="ps4", bufs=1, space="PSUM") as ps4,
        ):
            # --- head DMAs in FIFO order: chunk0 (first L1 input), then the
            # tiny weights/biases, then the remaining x chunks.
            xts = []
            for ci, (c0, c1) in enumerate(chunks):
                xt = xpool.tile([96, c1 - c0], F8, tag="x", name=f"x{ci}")
                xts.append(xt)
            nc.sync.dma_start(xts[0][:], xb[:, chunks[0][0]:chunks[0][1]])

            wpt = cpool.tile([128, 768], F8, tag="wp")
            nc.sync.dma_start(wpt[:], wp[:])
            w1bt = cpool.tile([96, 128], BF16, tag="w1b")
            nc.sync.dma_start(w1bt[:], w1b[:])
            w1t = w1bt[:]
            w2t = wpt[:, 0:256].rearrange("p (i m) -> p i m", i=2)
            w3t = wpt[:, 256:512].rearrange("p (i m) -> p i m", i=2)
            w4t = wpt[:, 512:768].rearrange("p (i m) -> p i m", i=2)

            bvt = cpool.tile([128, 4], F32, tag="bv")
            nc.sync.dma_start(bvt[:], bv[:])
            b1v, b2v, b3v, b9v = (bvt[:, 0:1], bvt[:, 1:2], bvt[:, 2:3],
                                  bvt[:, 3:4])

            for ci, (c0, c1) in enumerate(chunks):
                if ci > 0:
                    nc.sync.dma_start(xts[ci][:], xb[:, c0:c1])

            # PE warmup scratch (gpsimd memset, first instruction so the
            # warmup chain starts ASAP) + ACT relu-table preload (~1.3us,
            # overlaps head DMAs)
            wscr = cpool.tile([96, FD], BF16, tag="wscr")
            nc.gpsimd.memset(wscr[:], 0.0)
            tscr = cpool.tile([128, 8], F32, tag="tscr")
            nc.gpsimd.memset(tscr[:], 0.0)
            nc.scalar.activation(tscr[:], tscr[:], Relu, bias=0.0, scale=1.0)

            def filler():
                if FILL_MODE == "ldw":
                    for _ in range(FILL_PER_CHUNK):
                        nc.tensor.ldweights(wscr[:, 0:128])

            # head warmup: dependency-free high-duty matmul chain into
            # ps1's first tile (retired before the first real L1 claims a
            # buffer); 8-col stationary keeps LDWEIGHTS tiny.
            if warmup:
                wtile = ps1.tile([128, BLK], F32, tag="z1")
                for _ in range(warmup):
                    nc.tensor.matmul(wtile[0:WARMCOLS, 0:FD],
                                     wscr[:, 0:WARMCOLS],
                                     wscr[:], start=True, stop=True)

            def evac(dst, src, bias_ap, eng):
                if eng == "act":
                    nc.scalar.activation(dst, src, Relu, bias=bias_ap,
                                         scale=1.0)
                else:
                    nc.vector.tensor_scalar(dst, src, bias_ap, 0.0,
                                            ADD, MAX)

            def cevac(dst, src, eng):
                if eng == "act":
                    nc.scalar.activation(dst, src, Copy, bias=0.0, scale=1.0)
                else:
                    nc.vector.tensor_copy(dst, src)

            z1ps = {}
            z1rs = {}
            ps2ts = {}
            z2rs = {}
            ps3ts = {}
            z3rs = {}
            ps4ts = {}
            otile = None
            for w in range(n_ch + 8):
                # -- L1(w): two plain bf16-stationary x fp8-moving matmuls
                if w < n_ch:
                    ci, coff = chunk_of_block[w]
                    xt = xts[ci]
                    z1p = ps1.tile([128, BLK], F32, tag="z1")
                    for j in range(2):
                        nc.tensor.matmul(
                            z1p[:, j * FD : (j + 1) * FD],
                            w1t,
                            xt[:, coff + j * FD : coff + (j + 1) * FD],
                            start=True, stop=True,
                        )
                    z1ps[w] = z1p
                    if FILL_MODE != "off":
                        filler()

                # -- E1(w-1)
                c = w - 1
                if c in z1ps:
                    z1r = z1pool.tile([128, 2, FD], F8, tag="z1r")
                    z1src = z1ps.pop(c)
                    z1dst = z1r[:].rearrange("p i n -> p (i n)")
                    if c < 6 or c >= n_ch - 2:
                        # fill/drain waves: split across both engines
                        # (halves are separate banks) -- during fill only
                        # E1 work exists, so one engine would idle; at the
                        # drain it shortens the tail chain
                        evac(z1dst[:, 0:FD], z1src[:, 0:FD], b1v, "act")
                        evac(z1dst[:, FD:BLK], z1src[:, FD:BLK], b1v, "dve")
                        load["act"] += evac_cost(FD, "act")
                        load["dve"] += evac_cost(FD, "dve")
                    else:
                        evac(z1dst, z1src[:], b1v, pick_engine(BLK))
                    z1rs[c] = z1r

                # -- L2 pair k at w = 2k+4 (both chunks of the pair)
                if w >= 4 and (w - 4) % 2 == 0 and (w - 4) // 2 < n_pair:
                    k = (w - 4) // 2
                    ps2t = ps2.tile([128, BLK], F32, tag="z2")
                    for i in range(2):
                        nc.tensor.matmul(
                            ps2t[:, i * FD : (i + 1) * FD],
                            w2t,
                            z1rs.pop(2 * k + i)[:],
                            start=True, stop=True, perf_mode=DR,
                        )
                    ps2ts[k] = ps2t

                # -- E2 pair k at w = 2k+5
                if w >= 5 and (w - 5) % 2 == 0 and (w - 5) // 2 in ps2ts:
                    k = (w - 5) // 2
                    z2r = z2pool.tile([128, 2, FD], F8, tag="z2r")
                    z2dst = z2r[:].rearrange("p i n -> p (i n)")
                    z2src = ps2ts.pop(k)
                    if k == n_pair - 1:
                        # drain: split across engines (halves = sep banks)
                        evac(z2dst[:, 0:FD], z2src[:, 0:FD], b2v, "act")
                        evac(z2dst[:, FD:BLK], z2src[:, FD:BLK], b2v, "dve")
                    else:
                        evac(z2dst, z2src[:], b2v, pick_engine(BLK))
                    z2rs[k] = z2r

                # -- L3 pair k at w = 2k+6
                if w >= 6 and (w - 6) % 2 == 0 and (w - 6) // 2 in z2rs:
                    k = (w - 6) // 2
                    ps3t = ps3.tile([128, FD], F32, tag="z3")
                    nc.tensor.matmul(ps3t[:], w3t, z2rs.pop(k)[:],
                                     start=True, stop=True, perf_mode=DR)
                    ps3ts[k] = ps3t

                # -- E3 pair k at w = 2k+7 -> half k%2 of quad tile
                if w >= 7 and (w - 7) % 2 == 0 and (w - 7) // 2 in ps3ts:
                    k = (w - 7) // 2
                    if k % 2 == 0:
                        z3rs[k // 2] = z3pool.tile([128, 2, FD], F8,
                                                   tag="z3r",
                                                   name=f"z3r{k // 2}")
                    evac(z3rs[k // 2][:, k % 2, :], ps3ts.pop(k)[:], b3v,
                         pick_engine(FD))

                # -- L4 quad q at w = 4q+10
                if w >= 10 and (w - 10) % 4 == 0 and (w - 10) // 4 < n_quad:
                    q = (w - 10) // 4
                    ps4t = ps4.tile([128, FD], F32, tag="z4")
                    nc.tensor.matmul(ps4t[:], w4t, z3rs.pop(q)[:],
                                     start=True, stop=True, perf_mode=DR)
                    ps4ts[q] = ps4t

                # -- E4 quad q at w = 4q+11 (raw z4 -> fp16 copy) + out DMA
                if w >= 11 and (w - 11) % 4 == 0 and (w - 11) // 4 in ps4ts:
                    q = (w - 11) // 4
                    ps4t = ps4ts.pop(q)
                    if q % 4 == 0:
                        osz = min(4 * FD, out_cols - q * FD)
                        otile = opool.tile([128, osz], F16, tag="ot")
                    oo = (q % 4) * FD
                    cevac(otile[:, oo : oo + FD], ps4t[:], pick_engine(FD))
                    last_tile = (q - q % 4) * FD + otile.shape[1] == out_cols
                    if last_tile:
                        # drain the final tile piecewise so the out-DMA
                        # overlaps the tail instead of serializing after it
                        d0 = (q - q % 4) * FD
                        nc.sync.dma_start(outb[:, d0 + oo : d0 + oo + FD],
                                          otile[:, oo : oo + FD])
                    elif oo + FD == otile.shape[1]:
                        d0 = (q - q % 4) * FD
                        nc.sync.dma_start(
                            outb[:, d0 : d0 + otile.shape[1]], otile[:]
                        )

    if split_waits:
        _split_multi_waits(nc)
    return nc


# ---------------------------------------------------------------- host side
def _block_weights(W1, W7, W8, W9):
    """L1 stationary bf16 [96,128]; L2-L4 fp8 DoubleRow pairs [128,2,128]."""
    import ml_dtypes

    f8 = ml_dtypes.float8_e4m3fn
    W7q = W7.astype(f8).astype(np.float32)
    W8q = W8.astype(f8).astype(np.float32)
    W9q = W9.astype(f8).astype(np.float32)

    w1blk = np.zeros((96, 128), np.float32)
    for r in range(16):
        w1blk[r * 6:r * 6 + 6, r * 8:r * 8 + 8] = W1.T

    w2blk = np.zeros((128, 64), np.float32)
    for r in range(16):
        w2blk[r * 8:r * 8 + 8, r * 4:r * 4 + 4] = W7q.T
    w2d = np.zeros((128, 2, 128), np.float32)
    w2d[:, 0, 0:64] = w2blk
    w2d[:, 1, 64:128] = w2blk

    w3blk = np.zeros((128, 64), np.float32)
    for h in range(2):
        for r in range(16):
            w3blk[h * 64 + r * 4:h * 64 + r * 4 + 4,
                  h * 32 + r * 2:h * 32 + r * 2 + 2] = W8q.T
    w3d = np.zeros((128, 2, 128), np.float32)
    w3d[:, 0, 0:64] = w3blk
    w3d[:, 1, 64:128] = w3blk

    w4blk = np.zeros((128, 64), np.float32)
    for iw in range(2):
        for h in range(2):
            for r in range(16):
                w4blk[iw * 64 + h * 32 + r * 2:iw * 64 + h * 32 + r * 2 + 2,
                      iw * 32 + h * 16 + r] = W9q.T[:, 0]
    w4d = np.zeros((128, 2, 128), np.float32)
    w4d[:, 0, 0:64] = w4blk
    w4d[:, 1, 64:128] = w4blk
    return w1blk, w2d, w3d, w4d


def _host_pack(x, W1, b1, W7, b7, W8, b8, W9, b9):
    import ml_dtypes

    f8 = ml_dtypes.float8_e4m3fn
    bf = ml_dtypes.bfloat16
    w1blk, w2d, w3d, w4d = _block_weights(W1, W7, W8, W9)
    w1pack = w1blk.astype(bf)
    wpack = np.zeros((128, 768), np.float32)
    wpack[:, 0:256] = w2d.reshape(128, 256)
    wpack[:, 256:512] = w3d.reshape(128, 256)
    wpack[:, 512:768] = w4d.reshape(128, 256)
    wpack = wpack.astype(f8)
    bvecs = np.stack(
        [
            b1[np.arange(128) % 8],
            b7[np.arange(128) % 4],
            b8[np.arange(128) % 2],
            np.full(128, float(b9[0])),
        ],
        axis=1,
    ).astype(np.float32)
    # x [N,6] -> per core [96, COLS]: xpack[r*6+k, n] = x[16n + r, k]
    n = x.shape[0]
    cols = n // (N_CORES * G)
    xp = (
        x.reshape(N_CORES, cols, G, 6)
        .transpose(0, 2, 3, 1)
        .reshape(N_CORES, 96, cols)
        .astype(f8)
    )
    return np.ascontiguousarray(xp), w1pack, wpack, bvecs


def _out_row_index(out_cols):
    """row[q, col] for the [128, out_cols] output tile (per core).

    col = Q*512 + j; q = i*64 + iw*32 + h*16 + r; chunk c = 4Q + 2i + iw;
    group n = c*1024 + h*512 + j; row = 16n + r.
    """
    q = np.arange(128)[:, None]
    col = np.arange(out_cols)[None, :]
    Q = col >> 9
    j = col & 511
    i = q >> 6
    iw = (q >> 5) & 1
    h = (q >> 4) & 1
    r = q & 15
    n = (4 * Q + 2 * i + iw) * 1024 + h * 512 + j
    return (16 * n + r).astype(np.int64)


def _unpack_out(arr, n_blk=N_BLK):
    out_cols = arr.shape[1]
    rows = _out_row_index(out_cols)
    res = np.empty(out_cols * 128, np.float32)
    res[rows.ravel()] = np.asarray(arr, np.float32).ravel()
    return res.reshape(-1, 1)


def kernel(x, W1, b1, W7, b7, W8, b8, W9, b9):
    x = np.ascontiguousarray(np.asarray(x, dtype=np.float32))
    W1, b1 = np.asarray(W1, np.float32), np.asarray(b1, np.float32)
    W7, b7 = np.asarray(W7, np.float32), np.asarray(b7, np.float32)
    W8, b8 = np.asarray(W8, np.float32), np.asarray(b8, np.float32)
    W9, b9 = np.asarray(W9, np.float32), np.asarray(b9, np.float32)

    xp, w1pack, wpack, bvecs = _host_pack(x, W1, b1, W7, b7, W8, b8, W9, b9)

    nc = build_program()
    in_maps = [{"xb": xp[c], "w1b": w1pack, "wpack": wpack, "bvecs": bvecs}
               for c in range(N_CORES)]
    kwargs = {}
    if TRACE:
        _register_ntff_hook()
        kwargs["trace"] = True
    res = run_bass_kernel_spmd(nc, in_maps, list(range(N_CORES)), **kwargs)
    global LAST_RESULTS
    LAST_RESULTS = res

    outs = []
    for c in range(N_CORES):
        arr = np.asarray(res.results[c]["outb"], dtype=np.float32)
        outs.append(_unpack_out(arr))
    out = np.ascontiguousarray(np.concatenate(outs, axis=0))
    # device returned raw z4 (fp16); softplus(z4 + b9) on host
    out = np.logaddexp(0.0, out + float(b9[0])).astype(np.float32)
    return out


# revision 27
# speedup vs baseline: 1.2475x; 1.2475x over previous
"""Trainium2 Bass kernel for nn_Encoder (dense MLP 6->8->4->2->1 + softplus).

Pure data parallel over 8 NeuronCores. v2: every layer runs as an fp8e4m3
DoubleRow matmul (2 output cols/cycle, K-pairs contracted together), which
cuts PE matmul work from ~92k to ~31k cycles/core. All inter-layer
activations are fp8e4m3 (numpy-modeled rel_absmax 1.41e-2 vs 2e-2 budget;
HW fp8 rounding verified bit-identical to ml_dtypes in probe1.py).

The critical resource is PSUM->SBUF evacuation: only ACT (1.2 GHz) + DVE
(0.96 GHz) can read PSUM, 1 elem/lane/cycle (GPSIMD cannot access PSUM --
walrus birverifier; bf16 PSUM + DVE 2x is TRN3-only). 61440 evac columns
=> ~32us floor. Ops are sized to amortize per-op overhead (z1+z2 as
[128,1024]) and assigned to ACT/DVE by a build-time balancer.

Schedule (wave w, one 1024-group-col chunk per wave): L1(w) -> E1(w-1) ->
L2 pair k at 2k+4 -> E2 at 2k+5 -> L3 at 2k+6 -> E3 at 2k+7 -> L4 quad q
at 4q+10 -> E4(exp) at 4q+11; every cross-engine edge has >=1 wave slack.
PSUM: z1 [128,1024] double-buffered (4 banks) + z2 [128,1024] (2) +
z3 [128,512] + z4 [128,512] = 8 banks.

The PE HAM clock-gate (1.2 -> 2.4 GHz after ~3.4us of ~95% PE-busy) is
warmed with head matmuls + LDWEIGHTS fillers; with DoubleRow the PE has
so much slack that even a never-warm run stays near the evac floor.

Device computes exp(z4+b9) (no softplus ACT table in this toolchain),
ships bf16; host finishes softplus = log1p(exp) while unscrambling.
"""

import os
import sys

sys.path.insert(0, "/opt/trn_rl_repo")

import numpy as np

import concourse.bass as bass
import concourse.mybir as mybir
import concourse.tile as tile
from concourse.bass_utils import run_bass_kernel_spmd

# ---------------------------------------------------------------- geometry
N_CORES = 8
N_ROWS = 4194304
ROWS_PER_CORE = N_ROWS // N_CORES          # 524288
G = 16                                      # rows per group-column
COLS = ROWS_PER_CORE // G                   # 32768 group-cols per core
BLK = 1024                                  # group-cols per pipeline wave
N_BLK = COLS // BLK                         # 32
FD = 512                                    # matmul free dim / PSUM bank
OUT_COLS = COLS // 8                        # 4096 output columns [128p]
BF16 = mybir.dt.bfloat16
F8 = mybir.dt.float8e4
F32 = mybir.dt.float32

# walrus in this container rejects instructions carrying more than
# _MAX_WAITS sync waits; split the surplus onto same-engine NoOps placed
# immediately before the instruction.
_MAX_WAITS = int(os.environ.get("KMAXW", "1"))

# HAM warmup: the PE clock-gate opens after one 4096-cycle (3.4us @1.2GHz)
# window of ~fully-busy PE. v3 showed a chain with 128-col LDWEIGHTS between
# matmuls (80% duty) does NOT open it; use an 8-col stationary so the chain
# is ~96% matmul (8 b2b 512-col matmuls = 3.4us cold).
# 9 pure warm matmuls (3.9us @1.2GHz): the full 3.4us HAM window must be
# covered by the ~96%-duty chain alone -- mixing in real L1s (ldweights
# dilution) measurably fails to open the gate (v6: opened at 32us).
WARMUP = int(os.environ.get("KWARMUP", "9"))
WARMCOLS = int(os.environ.get("KWARMCOLS", "8"))
# PE filler mode: dependency-free LDWEIGHTS keep the PE busy so the HAM
# clock-gate opens. "ldw" / "off". With a >=3.4us back-to-back warmup the
# gate opens at the head and only re-throttles after >=3.4us of contiguous
# PE idle, which the steady-state pipeline never has -- fillers off.
FILL_MODE = os.environ.get("KFILLMODE", "off")
FILL_PER_CHUNK = int(os.environ.get("KFILLN", "2"))


def _split_multi_waits(nc, max_waits=_MAX_WAITS):
    ctr = 0
    for f in nc.m.functions:
        for bb in f.blocks:
            out = []
            for inst in bb.instructions:
                mw = 1 if ("Dma" in inst.opcode or "DMA" in inst.opcode
                           or "Trigger" in inst.opcode) else max_waits
                si = getattr(inst, "sync_info", None)
                if si is not None and si.on_wait and len(si.on_wait) > mw:
                    waits = list(si.on_wait)
                    split = len(waits) - mw
                    for i in range(0, split, max_waits):
                        nop = mybir.InstNoOp(
                            name=f"waitsplit-{ctr}", ins=[], outs=[]
                        )
                        ctr += 1
                        nop.engine = inst.engine
                        nop.sync_info = mybir.SyncInfo(
                            on_wait=waits[i : min(i + max_waits, split)],
                            on_update=[],
                        )
                        out.append(nop)
                    inst.sync_info = mybir.SyncInfo(
                        on_wait=waits[split:], on_update=list(si.on_update)
                    )
                out.append(inst)
            bb.instructions[:] = out


# Set KERNEL_TRACE=1 to neuron-profile the run; kernel() then stashes the
# BassKernelResults (exec_time_ns, trace paths) in LAST_RESULTS.
TRACE = os.environ.get("KERNEL_TRACE", "0") == "1"
LAST_RESULTS = None


def _register_ntff_hook():
    """The image's antenv lacks axon_hooks; inject it and register the ctypes
    NTFF profile hook so run_bass_kernel_spmd(trace=True) works under axon."""
    import types

    if "antenv.axon_hooks" not in sys.modules:
        mod = types.ModuleType("antenv.axon_hooks")
        mod._hook = None

        def set_axon_ntff_profile_hook(h, _mod=mod):
            _mod._hook = h

        def get_axon_ntff_profile_hook(_mod=mod):
            return _mod._hook

        mod.set_axon_ntff_profile_hook = set_axon_ntff_profile_hook
        mod.get_axon_ntff_profile_hook = get_axon_ntff_profile_hook
        sys.modules["antenv.axon_hooks"] = mod
        import antenv

        antenv.axon_hooks = mod
    mod = sys.modules["antenv.axon_hooks"]
    if mod.get_axon_ntff_profile_hook() is None:
        try:
            from trn_agent_boot.trn_boot import _ntff_profile_via_ctypes

            mod.set_axon_ntff_profile_hook(
                _ntff_profile_via_ctypes("/opt/axon/libaxon_pjrt.so")
            )
        except Exception:
            pass


# ---------------------------------------------------------------- program
def build_program(n_blk=N_BLK, split_waits=True, warmup=WARMUP):
    """One SPMD NeuronCore program; all 8 cores run it on their own shard."""
    nc = bass.Bass("TRN2", target_bir_lowering=False, debug=False,
                   num_devices=N_CORES)

    cols = n_blk * BLK
    out_cols = cols // 8
    n_ch = n_blk
    n_pair = n_ch // 2
    n_quad = n_ch // 4

    xb = nc.dram_tensor("xb", [96, cols], F8, kind="ExternalInput").ap()
    # L1 stationary stays bf16: fp8 gains nothing without DoubleRow (the
    # array runs 1 elem/cell/cycle regardless of width) and W1-fp8 is the
    # single biggest error term. Mixed bf16-stationary x fp8-moving keeps
    # FWL on.
    w1b = nc.dram_tensor("w1b", [96, 128], BF16, kind="ExternalInput").ap()
    # weight pack [128, 768] fp8: [:,0:256]=w2d, [:,256:512]=w3d,
    # [:,512:768]=w4d (each [128, 2, 128] DoubleRow pair)
    wp = nc.dram_tensor("wpack", [128, 768], F8, kind="ExternalInput").ap()
    bv = nc.dram_tensor("bvecs", [128, 4], F32, kind="ExternalInput").ap()
    # raw z4 ships as fp16 (rel err 5e-4); host computes the whole
    # softplus(z4+b9). Keeping exp off the device frees ~5us of
    # forced-ACT work into the ACT/DVE balance pool.
    outb = nc.dram_tensor("outb", [128, out_cols], mybir.dt.float16,
                          kind="ExternalOutput").ap()

    Relu = mybir.ActivationFunctionType.Relu
    Copy = mybir.ActivationFunctionType.Copy
    ADD = mybir.AluOpType.add
    MAX = mybir.AluOpType.max
    DR = mybir.MatmulPerfMode.DoubleRow
    F16 = mybir.dt.float16

    # x DMA chunks (in group-cols); first small so compute starts early
    if n_blk >= 16:
        chunks = [(0, 2048), (2048, 8192), (8192, 20480), (20480, cols)]
    else:
        chunks = [(i * BLK, (i + 1) * BLK) for i in range(n_blk)]
    chunk_of_block = []
    for i in range(n_ch):
        for ci, (c0, c1) in enumerate(chunks):
            if c0 <= i * BLK < c1:
                chunk_of_block.append((ci, i * BLK - c0))
                break

    # greedy ACT/DVE load balancer for the PSUM->SBUF evacuation ops.
    # per-op overheads measured from neuron-profile (v1): ACTIVATE +352cyc,
    # TENSOR_SCALAR +209cyc.
    load = {"act": 0.0, "dve": 0.0}
    act_oh = int(os.environ.get("KACTOH", "352"))
    dve_oh = int(os.environ.get("KDVEOH", "209"))

    def evac_cost(n, eng):
        return (n + act_oh) / 1.2 if eng == "act" else (n + dve_oh) / 0.96

    def pick_engine(n):
        a = load["act"] + evac_cost(n, "act")
        d = load["dve"] + evac_cost(n, "dve")
        eng = "act" if a <= d else "dve"
        load[eng] += evac_cost(n, eng)
        return eng

    with tile.TileContext(nc) as tc:
        with (
            tc.tile_pool(name="consts", bufs=1) as cpool,
            tc.tile_pool(name="xin", bufs=len(chunks)) as xpool,
            tc.tile_pool(name="z1r", bufs=5) as z1pool,
            tc.tile_pool(name="z2r", bufs=2) as z2pool,
            tc.tile_pool(name="z3r", bufs=2) as z3pool,
            tc.tile_pool(name="osb", bufs=2) as opool,
            tc.tile_pool(name="ps1", bufs=2, space="PSUM") as ps1,
            tc.tile_pool(name="ps2", bufs=1, space="PSUM") as ps2,
            tc.tile_pool(name="ps3", bufs=1, space="PSUM") as ps3,
            tc.tile_pool(name="ps4", bufs=1, space="PSUM") as ps4,
        ):
            # --- head DMAs in FIFO order: chunk0 (first L1 input), then the
            # tiny weights/biases, then the remaining x chunks.
            xts = []
            for ci, (c0, c1) in enumerate(chunks):
                xt = xpool.tile([96, c1 - c0], F8, tag="x", name=f"x{ci}")
                xts.append(xt)
            nc.sync.dma_start(xts[0][:], xb[:, chunks[0][0]:chunks[0][1]])

            wpt = cpool.tile([128, 768], F8, tag="wp")
            nc.sync.dma_start(wpt[:], wp[:])
            w1bt = cpool.tile([96, 128], BF16, tag="w1b")
            nc.sync.dma_start(w1bt[:], w1b[:])
            w1t = w1bt[:]
            w2t = wpt[:, 0:256].rearrange("p (i m) -> p i m", i=2)
            w3t = wpt[:, 256:512].rearrange("p (i m) -> p i m", i=2)
            w4t = wpt[:, 512:768].rearrange("p (i m) -> p i m", i=2)

            bvt = cpool.tile([128, 4], F32, tag="bv")
            nc.sync.dma_start(bvt[:], bv[:])
            b1v, b2v, b3v, b9v = (bvt[:, 0:1], bvt[:, 1:2], bvt[:, 2:3],
                                  bvt[:, 3:4])

            for ci, (c0, c1) in enumerate(chunks):
                if ci > 0:
                    nc.sync.dma_start(xts[ci][:], xb[:, c0:c1])

            # PE warmup scratch (gpsimd memset, first instruction so the
            # warmup chain starts ASAP) + ACT relu-table preload (~1.3us,
            # overlaps head DMAs)
            wscr = cpool.tile([96, FD], BF16, tag="wscr")
            nc.gpsimd.memset(wscr[:], 0.0)
            tscr = cpool.tile([128, 8], F32, tag="tscr")
            nc.gpsimd.memset(tscr[:], 0.0)
            nc.scalar.activation(tscr[:], tscr[:], Relu, bias=0.0, scale=1.0)

            def filler():
                if FILL_MODE == "ldw":
                    for _ in range(FILL_PER_CHUNK):
                        nc.tensor.ldweights(wscr[:, 0:128])

            # head warmup: dependency-free high-duty matmul chain into
            # ps1's first tile (retired before the first real L1 claims a
            # buffer); 8-col stationary keeps LDWEIGHTS tiny.
            if warmup:
                wtile = ps1.tile([128, BLK], F32, tag="z1")
                for _ in range(warmup):
                    nc.tensor.matmul(wtile[0:WARMCOLS, 0:FD],
                                     wscr[:, 0:WARMCOLS],
                                     wscr[:], start=True, stop=True)

            def evac(dst, src, bias_ap, eng):
                if eng == "act":
                    nc.scalar.activation(dst, src, Relu, bias=bias_ap,
                                         scale=1.0)
                else:
                    nc.vector.tensor_scalar(dst, src, bias_ap, 0.0,
                                            ADD, MAX)

            def cevac(dst, src, eng):
                if eng == "act":
                    nc.scalar.activation(dst, src, Copy, bias=0.0, scale=1.0)
                else:
                    nc.vector.tensor_copy(dst, src)

            z1ps = {}
            z1rs = {}
            ps2ts = {}
            z2rs = {}
            ps3ts = {}
            z3rs = {}
            ps4ts = {}
            otile = None
            for w in range(n_ch + 8):
                # -- L1(w): two plain bf16-stationary x fp8-moving matmuls
                if w < n_ch:
                    ci, coff = chunk_of_block[w]
                    xt = xts[ci]
                    z1p = ps1.tile([128, BLK], F32, tag="z1")
                    for j in range(2):
                        nc.tensor.matmul(
                            z1p[:, j * FD : (j + 1) * FD],
                            w1t,
                            xt[:, coff + j * FD : coff + (j + 1) * FD],
                            start=True, stop=True,
                        )
                    z1ps[w] = z1p
                    if FILL_MODE != "off":
                        filler()

                # -- E1(w-1)
                c = w - 1
                if c in z1ps:
                    z1r = z1pool.tile([128, 2, FD], F8, tag="z1r")
                    z1src = z1ps.pop(c)
                    z1dst = z1r[:].rearrange("p i n -> p (i n)")
                    if c < 6 or c >= n_ch - 2:
                        # fill/drain waves: split across both engines
                        # (halves are separate banks) -- during fill only
                        # E1 work exists, so one engine would idle; at the
                        # drain it shortens the tail chain
                        evac(z1dst[:, 0:FD], z1src[:, 0:FD], b1v, "act")
                        evac(z1dst[:, FD:BLK], z1src[:, FD:BLK], b1v, "dve")
                        load["act"] += evac_cost(FD, "act")
                        load["dve"] += evac_cost(FD, "dve")
                    else:
                        evac(z1dst, z1src[:], b1v, pick_engine(BLK))
                    z1rs[c] = z1r

                # -- L2 pair k at w = 2k+4 (both chunks of the pair)
                if w >= 4 and (w - 4) % 2 == 0 and (w - 4) // 2 < n_pair:
                    k = (w - 4) // 2
                    ps2t = ps2.tile([128, BLK], F32, tag="z2")
                    for i in range(2):
                        nc.tensor.matmul(
                            ps2t[:, i * FD : (i + 1) * FD],
                            w2t,
                            z1rs.pop(2 * k + i)[:],
                            start=True, stop=True, perf_mode=DR,
                        )
                    ps2ts[k] = ps2t

                # -- E2 pair k at w = 2k+5
                if w >= 5 and (w - 5) % 2 == 0 and (w - 5) // 2 in ps2ts:
                    k = (w - 5) // 2
                    z2r = z2pool.tile([128, 2, FD], F8, tag="z2r")
                    z2dst = z2r[:].rearrange("p i n -> p (i n)")
                    z2src = ps2ts.pop(k)
                    if k == n_pair - 1:
                        # drain: split across engines (halves = sep banks)
                        evac(z2dst[:, 0:FD], z2src[:, 0:FD], b2v, "act")
                        evac(z2dst[:, FD:BLK], z2src[:, FD:BLK], b2v, "dve")
                    else:
                        evac(z2dst, z2src[:], b2v, pick_engine(BLK))
                    z2rs[k] = z2r

                # -- L3 pair k at w = 2k+6
                if w >= 6 and (w - 6) % 2 == 0 and (w - 6) // 2 in z2rs:
                    k = (w - 6) // 2
                    ps3t = ps3.tile([128, FD], F32, tag="z3")
                    nc.tensor.matmul(ps3t[:], w3t, z2rs.pop(k)[:],
                                     start=True, stop=True, perf_mode=DR)
                    ps3ts[k] = ps3t

                # -- E3 pair k at w = 2k+7 -> half k%2 of quad tile
                if w >= 7 and (w - 7) % 2 == 0 and (w - 7) // 2 in ps3ts:
                    k = (w - 7) // 2
                    if k % 2 == 0:
                        z3rs[k // 2] = z3pool.tile([128, 2, FD], F8,
                                                   tag="z3r",
                                                   name=f"z3r{k // 2}")
                    evac(z3rs[k // 2][:, k % 2, :], ps3ts.pop(k)[:], b3v,
                         pick_engine(FD))

                # -- L4 quad q at w = 4q+10
                if w >= 10 and (w - 10) % 4 == 0 and (w - 10) // 4 < n_quad:
                    q = (w - 10) // 4
                    ps4t = ps4.tile([128, FD], F32, tag="z4")
                    nc.tensor.matmul(ps4t[:], w4t, z3rs.pop(q)[:],
                                     start=True, stop=True, perf_mode=DR)
                    ps4ts[q] = ps4t

                # -- E4 quad q at w = 4q+11 (raw z4 -> fp16 copy) + out DMA
                if w >= 11 and (w - 11) % 4 == 0 and (w - 11) // 4 in ps4ts:
                    q = (w - 11) // 4
                    ps4t = ps4ts.pop(q)
                    if q % 4 == 0:
                        osz = min(4 * FD, out_cols - q * FD)
                        otile = opool.tile([128, osz], F16, tag="ot")
                    oo = (q % 4) * FD
                    cevac(otile[:, oo : oo + FD], ps4t[:], pick_engine(FD))
                    last_tile = (q - q % 4) * FD + otile.shape[1] == out_cols
                    if last_tile:
                        # drain the final tile piecewise so the out-DMA
                        # overlaps the tail instead of serializing after it
                        d0 = (q - q % 4) * FD
                        nc.sync.dma_start(outb[:, d0 + oo : d0 + oo + FD],
                                          otile[:, oo : oo + FD])
                    elif oo + FD == otile.shape[1]:
                        d0 = (q - q % 4) * FD
                        nc.sync.dma_start(
                            outb[:, d0 : d0 + otile.shape[1]], otile[:]
                        )

    if split_waits:
        _split_multi_waits(nc)
    return nc


# ---------------------------------------------------------------- host side
def _block_weights(W1, W7, W8, W9):
    """L1 stationary bf16 [96,128]; L2-L4 fp8 DoubleRow pairs [128,2,128]."""
    import ml_dtypes

    f8 = ml_dtypes.float8_e4m3fn
    W7q = W7.astype(f8).astype(np.float32)
    W8q = W8.astype(f8).astype(np.float32)
    W9q = W9.astype(f8).astype(np.float32)

    w1blk = np.zeros((96, 128), np.float32)
    for r in range(16):
        w1blk[r * 6:r * 6 + 6, r * 8:r * 8 + 8] = W1.T

    w2blk = np.zeros((128, 64), np.float32)
    for r in range(16):
        w2blk[r * 8:r * 8 + 8, r * 4:r * 4 + 4] = W7q.T
    w2d = np.zeros((128, 2, 128), np.float32)
    w2d[:, 0, 0:64] = w2blk
    w2d[:, 1, 64:128] = w2blk

    w3blk = np.zeros((128, 64), np.float32)
    for h in range(2):
        for r in range(16):
            w3blk[h * 64 + r * 4:h * 64 + r * 4 + 4,
                  h * 32 + r * 2:h * 32 + r * 2 + 2] = W8q.T
    w3d = np.zeros((128, 2, 128), np.float32)
    w3d[:, 0, 0:64] = w3blk
    w3d[:, 1, 64:128] = w3blk

    w4blk = np.zeros((128, 64), np.float32)
    for iw in range(2):
        for h in range(2):
            for r in range(16):
                w4blk[iw * 64 + h * 32 + r * 2:iw * 64 + h * 32 + r * 2 + 2,
                      iw * 32 + h * 16 + r] = W9q.T[:, 0]
    w4d = np.zeros((128, 2, 128), np.float32)
    w4d[:, 0, 0:64] = w4blk
    w4d[:, 1, 64:128] = w4blk
    return w1blk, w2d, w3d, w4d


def _host_pack(x, W1, b1, W7, b7, W8, b8, W9, b9):
    import ml_dtypes

    f8 = ml_dtypes.float8_e4m3fn
    bf = ml_dtypes.bfloat16
    w1blk, w2d, w3d, w4d = _block_weights(W1, W7, W8, W9)
    w1pack = w1blk.astype(bf)
    wpack = np.zeros((128, 768), np.float32)
    wpack[:, 0:256] = w2d.reshape(128, 256)
    wpack[:, 256:512] = w3d.reshape(128, 256)
    wpack[:, 512:768] = w4d.reshape(128, 256)
    wpack = wpack.astype(f8)
    bvecs = np.stack(
        [
            b1[np.arange(128) % 8],
            b7[np.arange(128) % 4],
            b8[np.arange(128) % 2],
            np.full(128, float(b9[0])),
        ],
        axis=1,
    ).astype(np.float32)
    # x [N,6] -> per core [96, COLS]: xpack[r*6+k, n] = x[16n + r, k]
    n = x.shape[0]
    cols = n // (N_CORES * G)
    xp = (
        x.reshape(N_CORES, cols, G, 6)
        .transpose(0, 2, 3, 1)
        .reshape(N_CORES, 96, cols)
        .astype(f8)
    )
    return np.ascontiguousarray(xp), w1pack, wpack, bvecs


def _out_row_index(out_cols):
    """row[q, col] for the [128, out_cols] output tile (per core).

    col = Q*512 + j; q = i*64 + iw*32 + h*16 + r; chunk c = 4Q + 2i + iw;
    group n = c*1024 + h*512 + j; row = 16n + r.
    """
    q = np.arange(128)[:, None]
    col = np.arange(out_cols)[None, :]
    Q = col >> 9
    j = col & 511
    i = q >> 6
    iw = (q >> 5) & 1
    h = (q >> 4) & 1
    r = q & 15
    n = (4 * Q + 2 * i + iw) * 1024 + h * 512 + j
    return (16 * n + r).astype(np.int64)


def _unpack_out(arr, n_blk=N_BLK):
    out_cols = arr.shape[1]
    rows = _out_row_index(out_cols)
    res = np.empty(out_cols * 128, np.float32)
    res[rows.ravel()] = np.asarray(arr, np.float32).ravel()
    return res.reshape(-1, 1)


def kernel(x, W1, b1, W7, b7, W8, b8, W9, b9):
    x = np.ascontiguousarray(np.asarray(x, dtype=np.float32))
    W1, b1 = np.asarray(W1, np.float32), np.asarray(b1, np.float32)
    W7, b7 = np.asarray(W7, np.float32), np.asarray(b7, np.float32)
    W8, b8 = np.asarray(W8, np.float32), np.asarray(b8, np.float32)
    W9, b9 = np.asarray(W9, np.float32), np.asarray(b9, np.float32)

    xp, w1pack, wpack, bvecs = _host_pack(x, W1, b1, W7, b7, W8, b8, W9, b9)

    nc = build_program()
    in_maps = [{"xb": xp[c], "w1b": w1pack, "wpack": wpack, "bvecs": bvecs}
               for c in range(N_CORES)]
    kwargs = {}
    if TRACE:
        _register_ntff_hook()
        kwargs["trace"] = True
    res = run_bass_kernel_spmd(nc, in_maps, list(range(N_CORES)), **kwargs)
    global LAST_RESULTS
    LAST_RESULTS = res

    outs = []
    for c in range(N_CORES):
        arr = np.asarray(res.results[c]["outb"], dtype=np.float32)
        outs.append(_unpack_out(arr))
    out = np.ascontiguousarray(np.concatenate(outs, axis=0))
    # device returned raw z4 (fp16); softplus(z4 + b9) on host
    out = np.logaddexp(0.0, out + float(b9[0])).astype(np.float32)
    return out
